# revision 1
# baseline (speedup 1.0000x reference)
"""Trainium2 Bass kernel for AverageSpanExtractor (segment mean over spans).

Math note: the reference's masked softmax over all-ones logits reduces
exactly to a mean over the span tokens [start, end):
    out[b, n, :] = mean(sequence_tensor[b, start:end, :]),
    [start, end) = span_indices[b, n].
(Masked lanes get weight exp(-1e32-1) == 0 exactly; valid lanes get
exp(0)/width == 1/width, and gathered indices walk end-1 .. start.)

Strategy (8 cores, batch-parallel — one batch element per core):
  1. Block-local prefix sums: per 128-token block, an inclusive
     triangular fp32 matmul produces DRAM rows R[128k+1 .. 128k+128];
     R[0] = 0. No cross-block carry -> no serial chain. Loads ride the
     SP HWDGE ring, stores the ACT ring.
  2. Correction at consume time: C[x] = R[x] + Q[(x-1)>>7]; a span of
     width <= 32 crosses at most one block boundary, so
       span_sum = R[e] - R[s] + T[gb],  gb = (qe - qs) * qe  (0 => none)
     with T[k] = R[128k] the block-total table (T[0] = 0 neutral row).
  3. Per column j (8 spans per partition): two [128,1]-index indirect
     row-gathers fetch R[e], R[s] (one index per partition is what the
     HW indirect DMA supports). The correction needs only the 33-row T
     table: it is fetched once into SBUF, and per column a one-hot
     matrix (DVE compare + PE transpose) matmuls against it on the
     otherwise-idle tensor engine: corr = onehot @ T.
  4. Combine (CE - CS + corr) * (1/width) and store, per column, so
     DVE/PE/store pipeline behind the remaining gathers.
"""

import numpy as np

B, S, D = 8, 4096, 256
N_SPANS = 1024
P = 128
NBLK = S // P          # 32 blocks of 128 tokens
JG = N_SPANS // P      # 8 spans per partition
NT = 33                # rows of the block-total table (incl. zero row)

_cached_nc = None


def build_nc():
    import concourse.bass as bass
    import concourse.bacc as bacc
    import concourse.mybir as mybir
    from concourse.tile import TileContext
    from concourse.masks import make_upper_triangular, make_identity

    f32 = mybir.dt.float32
    f32r = mybir.dt.float32r
    i32 = mybir.dt.int32
    Alu = mybir.AluOpType

    # Bacc (not raw Bass): its compile() pass splits multi-wait sync into
    # event-semaphore chains and moves matmul waits onto ldweights — walrus
    # codegen rejects >1 sync wait per instruction otherwise.
    nc = bacc.Bacc(None, target_bir_lowering=False, debug=False, num_devices=B)
    seq = nc.declare_dram_parameter("seq", [S, D], f32, isOutput=False)
    spans = nc.declare_dram_parameter("spans", [P, 2 * JG], i32, isOutput=False)
    out = nc.declare_dram_parameter("out", [N_SPANS, D], f32, isOutput=True)

    with TileContext(nc) as tc:
        with (
            tc.tile_pool(name="const", bufs=1) as const_pool,
            tc.tile_pool(name="x", bufs=4) as x_pool,
            tc.tile_pool(name="c", bufs=3) as c_pool,
            tc.tile_pool(name="ps", bufs=4, space="PSUM") as ps_pool,
            tc.tile_pool(name="ohp", bufs=2, space="PSUM") as oh_pool,
            tc.tile_pool(name="crp", bufs=2, space="PSUM") as cr_pool,
            tc.tile_pool(name="misc", bufs=1) as misc_pool,
            tc.tile_pool(name="g", bufs=8) as g_pool,
            tc.tile_pool(name="res", bufs=2) as res_pool,
            tc.tile_pool(name="dram", bufs=1, space="DRAM") as d_pool,
        ):
            # DRAM scratch: block-local prefix rows; row 0 is all zeros.
            cum = d_pool.tile([S + 1, D], f32)

            # Stationary weights: upper-triangular ones (incl. diagonal)
            # => psum = triT.T @ x gives inclusive within-block prefixes.
            # Exact fp32: phase 1 is DMA-bound, so the 4-cycle/row matmul
            # costs ~nothing over fp32r and keeps rel err at ~2e-7.
            tri = const_pool.tile([P, P], f32)
            make_upper_triangular(nc, tri[:], val=1.0, diag=True)
            ident = const_pool.tile([P, P], f32)
            make_identity(nc, ident[:])

            zrow = const_pool.tile([1, D], f32)
            nc.vector.memset(zrow[:], 0.0)
            nc.scalar.dma_start(out=cum[0:1, :], in_=zrow[:])

            # --- phase 1: block-local prefix sums (fully parallel) ---
            # 8 token-blocks per DMA instruction: 1 MiB transfers sit at
            # ~341 GB/s vs ~220 GB/s for 32 separate 128 KiB ones (the
            # per-partition chunk stays 1 KiB either way; fewer
            # instructions amortize the HWDGE fixed cost).
            MBK = 8
            for gblk in range(NBLK // MBK):
                t0 = gblk * MBK * P
                bigx = x_pool.tile([P, MBK * D], f32)
                nc.sync.dma_start(
                    out=bigx[:],
                    in_=seq[t0 : t0 + MBK * P, :].rearrange(
                        "(m p) d -> p m d", p=P
                    ),
                )
                bigc = c_pool.tile([P, MBK * D], f32)
                for m in range(MBK):
                    ps = ps_pool.tile([P, D], f32)
                    nc.tensor.matmul(
                        out=ps[:], lhsT=tri[:], rhs=bigx[:, m * D : (m + 1) * D],
                        start=True, stop=True,
                    )
                    nc.vector.tensor_copy(
                        out=bigc[:, m * D : (m + 1) * D], in_=ps[:]
                    )
                half = MBK // 2
                nc.scalar.dma_start(
                    out=cum[1 + t0 : 1 + t0 + half * P, :].rearrange(
                        "(m p) d -> p m d", p=P
                    ),
                    in_=bigc[:, : half * D],
                )
                nc.scalar.dma_start(
                    out=cum[1 + t0 + half * P : 1 + t0 + MBK * P, :].rearrange(
                        "(m p) d -> p m d", p=P
                    ),
                    in_=bigc[:, half * D :],
                )

            # --- span index prep (independent of phase 1; overlaps) ---
            # V[p, 2j] = start of span 8p+j, V[p, 2j+1] = its exclusive end.
            V = misc_pool.tile([P, 2 * JG], i32)
            nc.scalar.dma_start(out=V[:], in_=spans[:])
            Sx = V[:, 0 : 2 * JG : 2]
            Ex = V[:, 1 : 2 * JG : 2]

            wi = misc_pool.tile([P, JG], i32)
            nc.vector.tensor_tensor(out=wi[:], in0=Ex, in1=Sx, op=Alu.subtract)
            wf = misc_pool.tile([P, JG], f32)
            nc.vector.tensor_copy(out=wf[:], in_=wi[:])
            wrec = misc_pool.tile([P, JG], f32)
            nc.vector.reciprocal(out=wrec[:], in_=wf[:])

            # Correction block index: gb = (qe - qs) * qe in 0..32,
            # qe = (e-1)>>7, qs = (s-1)>>7 (arithmetic shift; s=0 -> -1 ok).
            em1 = misc_pool.tile([P, JG], i32)
            nc.vector.tensor_scalar(
                out=em1[:], in0=Ex, scalar1=-1, scalar2=None, op0=Alu.add
            )
            qe = misc_pool.tile([P, JG], i32)
            nc.vector.tensor_scalar(
                out=qe[:], in0=em1[:], scalar1=7, scalar2=None,
                op0=Alu.arith_shift_right,
            )
            sm1 = misc_pool.tile([P, JG], i32)
            nc.vector.tensor_scalar(
                out=sm1[:], in0=Sx, scalar1=-1, scalar2=None, op0=Alu.add
            )
            qs = misc_pool.tile([P, JG], i32)
            nc.vector.tensor_scalar(
                out=qs[:], in0=sm1[:], scalar1=7, scalar2=None,
                op0=Alu.arith_shift_right,
            )
            dq = misc_pool.tile([P, JG], i32)
            nc.vector.tensor_tensor(out=dq[:], in0=qe[:], in1=qs[:], op=Alu.subtract)
            gb = misc_pool.tile([P, JG], i32)
            nc.vector.tensor_tensor(out=gb[:], in0=dq[:], in1=qe[:], op=Alu.mult)
            gbf = misc_pool.tile([P, JG], f32)
            nc.vector.tensor_copy(out=gbf[:], in_=gb[:])

            # iota 0..32 along free (every partition) for one-hot compares.
            iota33 = misc_pool.tile([P, NT], i32)
            nc.gpsimd.iota(iota33[:], pattern=[[1, NT]], base=0, channel_multiplier=0)
            iota33f = misc_pool.tile([P, NT], f32)
            nc.vector.tensor_copy(out=iota33f[:], in_=iota33[:])

            # Index column for the block-total fetch: ti[k] = 128*k.
            ti = misc_pool.tile([NT, 1], i32)
            nc.gpsimd.iota(ti[:], pattern=[[0, 1]], base=0, channel_multiplier=128)

            # --- phase 2: block-total table, then per-column pipeline ---
            Ttab = misc_pool.tile([NT, D], f32)
            nc.gpsimd.indirect_dma_start(
                out=Ttab[:], out_offset=None, in_=cum[:],
                in_offset=bass.IndirectOffsetOnAxis(ap=ti[:, 0:1], axis=0),
            )

            for j in range(JG):
                CE = g_pool.tile([P, D], f32)
                nc.gpsimd.indirect_dma_start(
                    out=CE[:], out_offset=None, in_=cum[:],
                    in_offset=bass.IndirectOffsetOnAxis(
                        ap=V[:, 2 * j + 1 : 2 * j + 2], axis=0
                    ),
                )
                CS = g_pool.tile([P, D], f32)
                nc.gpsimd.indirect_dma_start(
                    out=CS[:], out_offset=None, in_=cum[:],
                    in_offset=bass.IndirectOffsetOnAxis(
                        ap=V[:, 2 * j : 2 * j + 1], axis=0
                    ),
                )
                # One-hot correction on the otherwise-idle PE:
                # corr[p, :] = Ttab[gb[p, j], :]  (row 0 is zeros).
                ohT = misc_pool.tile([P, NT], f32, name=f"ohT{j}")
                nc.vector.tensor_tensor(
                    out=ohT[:], in0=iota33f[:],
                    in1=gbf[:, j : j + 1].to_broadcast([P, NT]),
                    op=Alu.is_equal,
                )
                ohp = oh_pool.tile([NT, P], f32)
                nc.tensor.transpose(out=ohp[:], in_=ohT[:], identity=ident[:])
                ohS = misc_pool.tile([NT, P], f32, name=f"ohS{j}")
                nc.vector.tensor_copy(out=ohS[:], in_=ohp[:])
                corr = cr_pool.tile([P, D], f32)
                nc.tensor.matmul(
                    out=corr[:], lhsT=ohS[:], rhs=Ttab[:], start=True, stop=True
                )

                rj = res_pool.tile([P, D], f32)
                nc.vector.tensor_tensor(out=rj[:], in0=CE[:], in1=CS[:], op=Alu.subtract)
                nc.vector.tensor_tensor(out=rj[:], in0=rj[:], in1=corr[:], op=Alu.add)
                nc.vector.tensor_scalar_mul(
                    out=rj[:], in0=rj[:], scalar1=wrec[:, j : j + 1]
                )
                # span at (p, j) is out row 8p+j.
                oj = out[:].rearrange("(p jj) d -> p jj d", p=P)[:, j, :]
                nc.scalar.dma_start(out=oj, in_=rj[:])
    nc.finalize()  # Bacc.finalize -> compile() (wait splitting) + freeze
    return nc


def _make_in_maps(sequence_tensor, span_indices):
    seq = np.ascontiguousarray(np.asarray(sequence_tensor), dtype=np.float32)
    si32 = np.asarray(span_indices).astype(np.int32)  # values < 4096: lossless
    assert seq.shape == (B, S, D) and si32.shape == (B, N_SPANS, 2)
    return [
        {
            "seq": seq[b],
            "spans": np.ascontiguousarray(si32[b].reshape(P, 2 * JG)),
        }
        for b in range(B)
    ]


def kernel(sequence_tensor, span_indices):
    from concourse.bass_utils import run_bass_kernel_spmd

    global _cached_nc
    if _cached_nc is None:
        _cached_nc = build_nc()
    in_maps = _make_in_maps(sequence_tensor, span_indices)
    res = run_bass_kernel_spmd(_cached_nc, in_maps, list(range(B)))
    return np.stack([res.results[b]["out"] for b in range(B)], axis=0)

